# revision 19
# baseline (speedup 1.0000x reference)
"""Trainium2 Bass kernel for nn_Attention_63075889709156 (joint img/text attention).

Strategy: tensor-parallel over heads across 8 NeuronCores (3 heads each).
Per core:
  A) QKV projections (fp16 matmuls, fp32 PSUM accum) for both streams,
     fused per-head RMSNorm (ACT Square+accum, ln/exp rsqrt) and RoPE
     (gains + rotation signs folded into host-prepared cos/sin tables,
     applied with fused scalar_tensor_tensor ops), PE transposes to get
     Q^T/K^T [hd, seq] fp16; V kept natural [seq, hd] bf16.
  B) Attention per (head, 512-query chunk): scores computed transposed
     [tk, tq] so ACT exp output feeds the P@V matmul directly; row-sums
     via an all-ones stationary matmul (gives broadcast for free);
     normalize with reciprocal_approx_fast; out^T [hd, tq] fp16.
  C) Output projections vs column-slices of Wo/Wadd -> per-core partial
     [seq, 3072] fp32, summed across cores on the host (all-reduce
     equivalent), biases added on host.

kernel(**inputs) takes the FULL unsharded inputs and returns the full
(img_out, enc_out) tuple, matching reference.reference().
"""
import os
import sys

sys.path.insert(0, "/opt/trn_rl_repo")

import numpy as np

B, S_IMG, S_ENC, D, H_TOT, HD = 1, 2048, 512, 3072, 24, 128
S_TOT = S_IMG + S_ENC  # 2560
NCORES = 8
HL = H_TOT // NCORES   # 3 heads per core
JW = HL * HD           # 384 output cols per matrix per core
NKK = D // 128         # 24 contraction tiles
NB_ENC = S_ENC // 128  # 4
NB_IMG = S_IMG // 128  # 16
NB = NB_ENC + NB_IMG   # 20
CH = 512               # query-chunk / free-dim tile
NCHUNK = S_TOT // CH   # 5 (chunk 0 == the encoder tokens exactly)
MCH = D // CH          # 6 output column chunks
EPS = 1e-5
SCALE = 1.0 / float(np.sqrt(HD))

_CACHE = {}


def _build():
    import concourse.bass as bass
    import concourse.mybir as mybir
    import concourse.tile as tile
    from concourse import bacc
    from concourse.masks import make_identity

    dt = mybir.dt
    ts = bass.ts
    mult = mybir.AluOpType.mult
    AT = mybir.AluOpType
    MAGIC1 = 0x5F3759DF + 1
    AF = mybir.ActivationFunctionType

    nc = bacc.Bacc(trn_type="TRN2", target_bir_lowering=False)

    hT = nc.dram_tensor("hT", [D, S_IMG], dt.float16, kind="ExternalInput")
    eT = nc.dram_tensor("eT", [D, S_ENC], dt.float16, kind="ExternalInput")
    wqkv = nc.dram_tensor("wqkv", [D, 3 * JW], dt.float16, kind="ExternalInput")
    waqkv = nc.dram_tensor("waqkv", [D, 3 * JW], dt.float16, kind="ExternalInput")
    bqkv = nc.dram_tensor("bqkv", [1, 3 * JW], dt.float16, kind="ExternalInput")
    cosq = nc.dram_tensor("cosq", [S_TOT, HD], dt.float16, kind="ExternalInput")
    sinq = nc.dram_tensor("sinq", [S_TOT, HD], dt.float16, kind="ExternalInput")
    cosk = nc.dram_tensor("cosk", [S_TOT, HD], dt.float16, kind="ExternalInput")
    sink = nc.dram_tensor("sink", [S_TOT, HD], dt.float16, kind="ExternalInput")
    wo = nc.dram_tensor("wo", [JW, D], dt.float16, kind="ExternalInput")
    wadd = nc.dram_tensor("wadd", [JW, D], dt.float16, kind="ExternalInput")

    img_p = nc.dram_tensor("img_p", [S_IMG, D], dt.float16, kind="ExternalOutput")
    enc_p = nc.dram_tensor("enc_p", [S_ENC, D], dt.float16, kind="ExternalOutput")

    # partition-tiled DRAM views: [(kk p) n] -> [p kk n]
    hT_r = hT.rearrange("(kk p) t -> p kk t", p=128)
    eT_r = eT.rearrange("(kk p) t -> p kk t", p=128)
    wqkv_r = wqkv.rearrange("(kk p) j -> p kk j", p=128)
    waqkv_r = waqkv.rearrange("(kk p) j -> p kk j", p=128)
    wo_r = wo.rearrange("(j p) m -> p j m", p=128)      # [128, 3, 3072]
    wadd_r = wadd.rearrange("(j p) m -> p j m", p=128)

    with tile.TileContext(nc) as tc:
        with (
            tc.tile_pool(name="cpool", bufs=1) as cpool,
            tc.tile_pool(name="rpool", bufs=1) as rpool,
            tc.tile_pool(name="wpool", bufs=2) as wpool,
            tc.tile_pool(name="xpool", bufs=3) as xpool,
            tc.tile_pool(name="tpool", bufs=2) as tpool,
            tc.tile_pool(name="apool", bufs=3) as apool,
            tc.tile_pool(name="bpool", bufs=4) as bpool,
            tc.tile_pool(name="opool", bufs=3) as opool,
        ):
            # ---- constants ----
            id16 = cpool.tile([128, 128], dt.float16, tag="id16")
            make_identity(nc, id16[:])
            ones1 = cpool.tile([1, 128], dt.float16, tag="ones1")
            nc.vector.memset(ones1[:], 1.0)
            onesbf = cpool.tile([128, 128], dt.bfloat16, tag="onesbf")
            nc.vector.memset(onesbf[:], 1.0)
            bq_sb = cpool.tile([1, 3 * JW], dt.float16, tag="bq")
            nc.sync.dma_start(bq_sb[:], bqkv[:])

            # ---- resident tensors ----
            QT = [rpool.tile([128, S_TOT], dt.float16, tag=f"qt{h}", name=f"qt{h}") for h in range(HL)]
            KT = [rpool.tile([128, S_TOT], dt.float16, tag=f"kt{h}", name=f"kt{h}") for h in range(HL)]
            OT = [rpool.tile([128, S_TOT], dt.float16, tag=f"ot{h}", name=f"ot{h}") for h in range(HL)]
            v_sb = rpool.tile([128, NB * 3 * HD], dt.bfloat16, tag="vsb")
            v_v = v_sb[:].rearrange("p (b j) -> p b j", b=NB)  # [128, 20, 384]

            streams = [
                (NB_ENC, 0, eT_r, waqkv_r, True),
                (NB_IMG, NB_ENC, hT_r, wqkv_r, False),
            ]

            # ================= Phase A (own psum pool: 6 banks) ==========
            psA_ctx = tc.tile_pool(name="psA", bufs=2, space="PSUM")
            pspool = psA_ctx.__enter__()
            # HAM warmup: keep PE busy during the initial weight/act DMAs so
            # the clock gate reaches 2.4 GHz before the first real matmul.
            warm = pspool.tile([128, CH], dt.float32, tag="psC")
            for _ in range(40):
                nc.tensor.matmul(warm[:, 0:128], id16[:], id16[:],
                                 start=True, stop=True)
            # ---- Phase A-qk ----
            for nblk, blk0, x_r, w_r, has_bias in streams:
                w_t = wpool.tile([128, NKK * 2 * JW], dt.float16, tag="w")
                w_v = w_t[:].rearrange("p (kk j) -> p kk j", kk=NKK)
                nc.sync.dma_start(w_v[:, 0:6, :], w_r[:, 0:6, 0 : 2 * JW])
                for tb in range(nblk):
                    gtb = blk0 + tb
                    xt = xpool.tile([128, NKK * 128], dt.float16, tag="xt")
                    xt_v = xt[:].rearrange("p (kk t) -> p kk t", kk=NKK)
                    nc.sync.dma_start(xt_v, x_r[:, :, tb * 128 : tb * 128 + 128])
                    rsl = slice(gtb * 128, gtb * 128 + 128)
                    cq = tpool.tile([128, HD], dt.float16, tag="cq")
                    nc.sync.dma_start(cq[:], cosq[rsl, :])
                    sq_ = tpool.tile([128, HD], dt.float16, tag="sq")
                    nc.sync.dma_start(sq_[:], sinq[rsl, :])
                    ck = tpool.tile([128, HD], dt.float16, tag="ck")
                    nc.sync.dma_start(ck[:], cosk[rsl, :])
                    sk = tpool.tile([128, HD], dt.float16, tag="sk")
                    nc.sync.dma_start(sk[:], sink[rsl, :])
                    if tb == 0:
                        for g in range(1, 4):
                            gs = slice(g * 6, (g + 1) * 6)
                            nc.sync.dma_start(w_v[:, gs, :], w_r[:, gs, 0 : 2 * JW])

                    pq = pspool.tile([128, CH], dt.float32, tag="psA")
                    pk = pspool.tile([128, CH], dt.float32, tag="psB")
                    for kk in range(NKK):
                        st_f = kk == 0
                        sp_f = (kk == NKK - 1) and not has_bias
                        nc.tensor.matmul(pq[:, 0:JW], xt_v[:, kk], w_v[:, kk, 0:JW],
                                         start=st_f, stop=sp_f)
                        nc.tensor.matmul(pk[:, 0:JW], xt_v[:, kk], w_v[:, kk, JW : 2 * JW],
                                         start=st_f, stop=sp_f)
                    if has_bias:
                        nc.tensor.matmul(pq[:, 0:JW], ones1[:], bq_sb[:, 0:JW],
                                         start=False, stop=True)
                        nc.tensor.matmul(pk[:, 0:JW], ones1[:], bq_sb[:, JW : 2 * JW],
                                         start=False, stop=True)

                    qsb = apool.tile([128, JW], dt.float32, tag="qsb")
                    nc.scalar.copy(qsb[:], pq[:, 0:JW])
                    ksb = apool.tile([128, JW], dt.float32, tag="ksb")
                    nc.scalar.copy(ksb[:], pk[:, 0:JW])

                    junk = apool.tile([128, HD], dt.float32, tag="junk")
                    ssq = apool.tile([128, 2 * HL], dt.float32, tag="ssq")
                    for h in range(HL):
                        nc.scalar.activation(junk[:], pq[:, ts(h, HD)], AF.Square,
                                             accum_out=ssq[:, h : h + 1])
                    for h in range(HL):
                        nc.scalar.activation(junk[:], pk[:, ts(h, HD)], AF.Square,
                                             accum_out=ssq[:, HL + h : HL + h + 1])
                    # inv = rsqrt(ssq/HD + eps): bit-trick seed + 2 Newton iters
                    ms = apool.tile([128, 2 * HL], dt.float32, tag="ms")
                    nc.vector.tensor_scalar(ms[:], ssq[:], 1.0 / HD, EPS,
                                            op0=mult, op1=AT.add)
                    sh_t = apool.tile([128, 2 * HL], dt.int32, tag="sh_t")
                    nc.vector.tensor_scalar(sh_t[:], ms[:].bitcast(dt.int32), 1, -1,
                                            op0=AT.logical_shift_right,
                                            op1=AT.bitwise_xor)
                    ya = apool.tile([128, 2 * HL], dt.float32, tag="ya")
                    nc.vector.tensor_scalar(ya[:].bitcast(dt.int32), sh_t[:],
                                            MAGIC1, None, op0=AT.add)
                    nta = apool.tile([128, 2 * HL], dt.float32, tag="nta")
                    ntc = apool.tile([128, 2 * HL], dt.float32, tag="ntc")
                    inv = apool.tile([128, 2 * HL], dt.float32, tag="inv")
                    for ysrc, ydst in ((ya, inv), (inv, ya)):
                        nc.vector.tensor_mul(nta[:], ysrc[:], ysrc[:])
                        nc.vector.tensor_mul(nta[:], nta[:], ms[:])
                        nc.vector.tensor_scalar(ntc[:], nta[:], -0.5, 1.5,
                                                op0=mult, op1=AT.add)
                        nc.vector.tensor_mul(ydst[:], ysrc[:], ntc[:])
                    inv = ya

                    for src, dstT, ct, st_t, iv0 in (
                        (qsb, QT, cq, sq_, 0),
                        (ksb, KT, ck, sk, HL),
                    ):
                        t1 = apool.tile([128, JW], dt.float32, tag="t1")
                        t2 = apool.tile([128, JW], dt.float32, tag="t2")
                        st_p = st_t[:].rearrange("p (a b) -> p a b", b=2)
                        for h in range(HL):
                            iv = inv[:, iv0 + h : iv0 + h + 1]
                            hs = slice(h * HD, (h + 1) * HD)
                            nc.vector.scalar_tensor_tensor(
                                t1[:, hs], src[:, hs], iv, ct[:], op0=mult, op1=mult)
                            srev = src[:, hs].rearrange(
                                "p (a b) -> p a b", b=2)[:, :, ::-1]
                            t2p = t2[:, hs].rearrange("p (a b) -> p a b", b=2)
                            nc.vector.scalar_tensor_tensor(
                                t2p, srev, iv, st_p, op0=mult, op1=mult)
                        qf = apool.tile([128, JW], dt.float16, tag="qf")
                        nc.vector.tensor_add(qf[:], t1[:], t2[:])
                        for h in range(HL):
                            ptr = pspool.tile([128, CH], dt.float16, tag="psC")
                            nc.tensor.transpose(ptr[:, 0:128], qf[:, ts(h, HD)], id16[:])
                            nc.any.tensor_copy(dstT[h][:, gtb * 128 : gtb * 128 + 128],
                                               ptr[:, 0:128])

            # ---- Phase A-v ----
            for nblk, blk0, x_r, w_r, has_bias in streams:
                wv_t = wpool.tile([128, NKK * JW], dt.float16, tag="w")
                wv_v = wv_t[:].rearrange("p (kk j) -> p kk j", kk=NKK)
                nc.sync.dma_start(wv_v, w_r[:, :, 2 * JW : 3 * JW])
                for tb in range(nblk):
                    gtb = blk0 + tb
                    xt = xpool.tile([128, NKK * 128], dt.float16, tag="xt")
                    xt_v = xt[:].rearrange("p (kk t) -> p kk t", kk=NKK)
                    nc.sync.dma_start(xt_v, x_r[:, :, tb * 128 : tb * 128 + 128])
                    pv = pspool.tile([128, CH], dt.float32, tag="psA")  # shares pq slots
                    for kk in range(NKK):
                        nc.tensor.matmul(pv[:, 0:JW], xt_v[:, kk], wv_v[:, kk],
                                         start=(kk == 0),
                                         stop=(kk == NKK - 1) and not has_bias)
                    if has_bias:
                        nc.tensor.matmul(pv[:, 0:JW], ones1[:], bq_sb[:, 2 * JW : 3 * JW],
                                         start=False, stop=True)
                    nc.vector.tensor_copy(v_v[:, gtb, :], pv[:, 0:JW])

            psA_ctx.__exit__(None, None, None)

            # ============ Phases B + C (own psum pool: 8 banks) ==========
            psB_ctx = tc.tile_pool(name="psB", bufs=1, space="PSUM")
            pspool = psB_ctx.__enter__()
            wo_t = wpool.tile([128, HL * D], dt.float16, tag="w")
            wo_v = wo_t[:].rearrange("p (j m) -> p j m", j=HL)
            nc.sync.dma_start(wo_v, wo_r[:])
            wadd_t = wpool.tile([128, HL * D], dt.float16, tag="w")
            wadd_v = wadd_t[:].rearrange("p (j m) -> p j m", j=HL)
            nc.sync.dma_start(wadd_v, wadd_r[:])

            for chunk in range(NCHUNK):
                qsl = slice(chunk * CH, (chunk + 1) * CH)
                # all 3 heads interleaved: 3 independent ST->exp->AV/SM
                # streams keep PE and ACT decoupled. Head 2's accumulators
                # borrow the C-phase "po" slots (released before C runs).
                us, sms = [], []
                for h in range(HL):
                    utag = "u" if h < 2 else "po"
                    smtag = "sm" if h < 2 else "po"
                    u = pspool.tile([128, CH], dt.float32, tag=utag, bufs=2,
                                    name=f"u_{chunk}_{h}")
                    sm = pspool.tile([128, CH], dt.float32, tag=smtag, bufs=2,
                                     name=f"sm_{chunk}_{h}")
                    us.append(u)
                    sms.append(sm)
                for b in range(NB):
                    for h in range(HL):
                        st_ = pspool.tile([128, CH], dt.float32, tag="st", bufs=2)
                        pt = bpool.tile([128, CH], dt.bfloat16, tag="pt")
                        nc.tensor.matmul(st_[:], KT[h][:, ts(b, 128)],
                                         QT[h][:, qsl], start=True, stop=True)
                        nc.scalar.activation(pt[:], st_[:], AF.Exp, scale=SCALE)
                        nc.tensor.matmul(us[h][:], v_v[:, b, ts(h, HD)], pt[:],
                                         start=(b == 0), stop=(b == NB - 1))
                        nc.tensor.matmul(sms[h][:], onesbf[:], pt[:],
                                         start=(b == 0), stop=(b == NB - 1))
                for h in range(HL):
                    invb = bpool.tile([128, CH], dt.float32, tag="invb")
                    nc.vector.reciprocal_approx_fast(invb[:], sms[h][:])
                    nc.vector.tensor_mul(OT[h][:, qsl], us[h][:], invb[:])

                # output projection for this chunk's 4 token blocks
                w_sel = wadd_v if chunk == 0 else wo_v
                for tb in range(4):
                    gtb = chunk * 4 + tb
                    csl = slice(gtb * 128, gtb * 128 + 128)
                    for mc in range(MCH):
                        po = pspool.tile([128, CH], dt.float32, tag="po", bufs=2)
                        for j in range(HL):
                            nc.tensor.matmul(po[:], OT[j][:, csl],
                                             w_sel[:, j, ts(mc, CH)],
                                             start=(j == 0), stop=(j == HL - 1))
                        osb = opool.tile([128, CH], dt.float16, tag="osb")
                        nc.any.tensor_copy(osb[:], po[:])
                        if chunk == 0:
                            nc.sync.dma_start(enc_p[csl, ts(mc, CH)], osb[:])
                        else:
                            isl = slice(gtb * 128 - S_ENC, gtb * 128 - S_ENC + 128)
                            nc.sync.dma_start(img_p[isl, ts(mc, CH)], osb[:])

            psB_ctx.__exit__(None, None, None)

    nc.finalize()
    return nc


def _get_program():
    if "nc" not in _CACHE:
        _CACHE["nc"] = _build()
    return _CACHE["nc"]


def _prep_tables(cos, sin, g_enc, g_img):
    """Fold per-head gains and rotation signs into cos/sin tables.

    out = (x*inv*g)*cos + rot(x*inv*g)*sin with rot(x)[2i] = -x[2i+1],
    rot(x)[2i+1] = x[2i].  We apply (x*inv) on device, so:
      cos_t[t, d]   = cos[t, d] * g[d]
      sin_t[t, 2i]  = -sin[t, 2i]   * g[2i+1]   (multiplies x[2i+1])
      sin_t[t, 2i+1] = sin[t, 2i+1] * g[2i]     (multiplies x[2i])
    g = g_enc for t < S_ENC else g_img.
    """
    g = np.empty((S_TOT, HD), np.float32)
    g[:S_ENC] = np.asarray(g_enc, np.float32)[None, :]
    g[S_ENC:] = np.asarray(g_img, np.float32)[None, :]
    cos_t = (np.asarray(cos, np.float32) * g).astype(np.float32)
    sin_t = np.empty((S_TOT, HD), np.float32)
    sin_t[:, 0::2] = -np.asarray(sin, np.float32)[:, 0::2] * g[:, 1::2]
    sin_t[:, 1::2] = np.asarray(sin, np.float32)[:, 1::2] * g[:, 0::2]
    return (np.ascontiguousarray(cos_t).astype(np.float16),
            np.ascontiguousarray(sin_t).astype(np.float16))


def kernel(hidden_states, encoder_hidden_states, cos, sin,
           Wq, Wk, Wv, gq, gk,
           Waq, baq, Wak, bak, Wav, bav, gaq, gak,
           Wo, bo, Wadd, badd):
    from concourse.bass_utils import run_bass_kernel_spmd

    f32 = np.float32
    h = np.asarray(hidden_states, f32).reshape(S_IMG, D)
    e = np.asarray(encoder_hidden_states, f32).reshape(S_ENC, D)
    hT16 = np.ascontiguousarray(h.T).astype(np.float16)
    eT16 = np.ascontiguousarray(e.T).astype(np.float16)

    cos_q, sin_q = _prep_tables(cos, sin, gaq, gq)
    cos_k, sin_k = _prep_tables(cos, sin, gak, gk)

    Wq, Wk, Wv = (np.asarray(x, f32) for x in (Wq, Wk, Wv))
    Waq, Wak, Wav = (np.asarray(x, f32) for x in (Waq, Wak, Wav))
    Wo_, Wadd_ = np.asarray(Wo, f32), np.asarray(Wadd, f32)
    baq, bak, bav = (np.asarray(x, f32) for x in (baq, bak, bav))

    in_maps = []
    for c in range(NCORES):
        J = slice(c * JW, (c + 1) * JW)
        wqkv_c = np.concatenate([Wq[J].T, Wk[J].T, Wv[J].T], axis=1).astype(np.float16)
        waqkv_c = np.concatenate([Waq[J].T, Wak[J].T, Wav[J].T], axis=1).astype(np.float16)
        bqkv_c = np.concatenate([baq[J], bak[J], bav[J]])[None, :].astype(np.float16)
        wo_c = np.ascontiguousarray(Wo_[:, J].T).astype(np.float16)
        wadd_c = np.ascontiguousarray(Wadd_[:, J].T).astype(np.float16)
        in_maps.append({
            "hT": hT16, "eT": eT16,
            "wqkv": np.ascontiguousarray(wqkv_c),
            "waqkv": np.ascontiguousarray(waqkv_c),
            "bqkv": bqkv_c,
            "cosq": cos_q, "sinq": sin_q, "cosk": cos_k, "sink": sin_k,
            "wo": wo_c, "wadd": wadd_c,
        })

    nc = _get_program()
    trace = bool(int(os.environ.get("KERNEL_TRACE", "0")))
    res = run_bass_kernel_spmd(nc, in_maps, core_ids=list(range(NCORES)),
                               trace=trace)
    _CACHE["last_results"] = res

    img = np.zeros((S_IMG, D), f32)
    enc = np.zeros((S_ENC, D), f32)
    for c in range(NCORES):
        img += res.results[c]["img_p"]
        enc += res.results[c]["enc_p"]
    img += np.asarray(bo, f32)[None, :]
    enc += np.asarray(badd, f32)[None, :]
    return img.reshape(B, S_IMG, D), enc.reshape(B, S_ENC, D)


# revision 20
# speedup vs baseline: 1.0368x; 1.0368x over previous
"""Trainium2 Bass kernel for nn_Attention_63075889709156 (joint img/text attention).

Strategy: tensor-parallel over heads across 8 NeuronCores (3 heads each).
Per core:
  A) QKV projections (fp16 matmuls, fp32 PSUM accum) for both streams,
     fused per-head RMSNorm (ACT Square+accum, ln/exp rsqrt) and RoPE
     (gains + rotation signs folded into host-prepared cos/sin tables,
     applied with fused scalar_tensor_tensor ops), PE transposes to get
     Q^T/K^T [hd, seq] fp16; V kept natural [seq, hd] bf16.
  B) Attention per (head, 512-query chunk): scores computed transposed
     [tk, tq] so ACT exp output feeds the P@V matmul directly; row-sums
     via an all-ones stationary matmul (gives broadcast for free);
     normalize with reciprocal_approx_fast; out^T [hd, tq] fp16.
  C) Output projections vs column-slices of Wo/Wadd -> per-core partial
     [seq, 3072] fp32, summed across cores on the host (all-reduce
     equivalent), biases added on host.

kernel(**inputs) takes the FULL unsharded inputs and returns the full
(img_out, enc_out) tuple, matching reference.reference().
"""
import os
import sys

sys.path.insert(0, "/opt/trn_rl_repo")

import numpy as np

B, S_IMG, S_ENC, D, H_TOT, HD = 1, 2048, 512, 3072, 24, 128
S_TOT = S_IMG + S_ENC  # 2560
NCORES = 8
HL = H_TOT // NCORES   # 3 heads per core
JW = HL * HD           # 384 output cols per matrix per core
NKK = D // 128         # 24 contraction tiles
NB_ENC = S_ENC // 128  # 4
NB_IMG = S_IMG // 128  # 16
NB = NB_ENC + NB_IMG   # 20
CH = 512               # query-chunk / free-dim tile
NCHUNK = S_TOT // CH   # 5 (chunk 0 == the encoder tokens exactly)
MCH = D // CH          # 6 output column chunks
EPS = 1e-5
SCALE = 1.0 / float(np.sqrt(HD))

_CACHE = {}


def _build():
    import concourse.bass as bass
    import concourse.mybir as mybir
    import concourse.tile as tile
    from concourse import bacc
    from concourse.masks import make_identity

    dt = mybir.dt
    ts = bass.ts
    mult = mybir.AluOpType.mult
    AT = mybir.AluOpType
    MAGIC1 = 0x5F3759DF + 1
    AF = mybir.ActivationFunctionType

    nc = bacc.Bacc(trn_type="TRN2", target_bir_lowering=False)

    hT = nc.dram_tensor("hT", [D, S_IMG], dt.float16, kind="ExternalInput")
    eT = nc.dram_tensor("eT", [D, S_ENC], dt.float16, kind="ExternalInput")
    wqkv = nc.dram_tensor("wqkv", [D, 3 * JW], dt.float16, kind="ExternalInput")
    waqkv = nc.dram_tensor("waqkv", [D, 3 * JW], dt.float16, kind="ExternalInput")
    bqkv = nc.dram_tensor("bqkv", [1, 3 * JW], dt.float16, kind="ExternalInput")
    cosq = nc.dram_tensor("cosq", [S_TOT, HD], dt.float16, kind="ExternalInput")
    sinq = nc.dram_tensor("sinq", [S_TOT, HD], dt.float16, kind="ExternalInput")
    cosk = nc.dram_tensor("cosk", [S_TOT, HD], dt.float16, kind="ExternalInput")
    sink = nc.dram_tensor("sink", [S_TOT, HD], dt.float16, kind="ExternalInput")
    wo = nc.dram_tensor("wo", [JW, D], dt.float16, kind="ExternalInput")
    wadd = nc.dram_tensor("wadd", [JW, D], dt.float16, kind="ExternalInput")

    img_p = nc.dram_tensor("img_p", [S_IMG, D], dt.float16, kind="ExternalOutput")
    enc_p = nc.dram_tensor("enc_p", [S_ENC, D], dt.float16, kind="ExternalOutput")

    # partition-tiled DRAM views: [(kk p) n] -> [p kk n]
    hT_r = hT.rearrange("(kk p) t -> p kk t", p=128)
    eT_r = eT.rearrange("(kk p) t -> p kk t", p=128)
    wqkv_r = wqkv.rearrange("(kk p) j -> p kk j", p=128)
    waqkv_r = waqkv.rearrange("(kk p) j -> p kk j", p=128)
    wo_r = wo.rearrange("(j p) m -> p j m", p=128)      # [128, 3, 3072]
    wadd_r = wadd.rearrange("(j p) m -> p j m", p=128)

    with tile.TileContext(nc) as tc:
        with (
            tc.tile_pool(name="cpool", bufs=1) as cpool,
            tc.tile_pool(name="rpool", bufs=1) as rpool,
            tc.tile_pool(name="wpool", bufs=2) as wpool,
            tc.tile_pool(name="xpool", bufs=3) as xpool,
            tc.tile_pool(name="tpool", bufs=2) as tpool,
            tc.tile_pool(name="apool", bufs=3) as apool,
            tc.tile_pool(name="bpool", bufs=4) as bpool,
            tc.tile_pool(name="opool", bufs=3) as opool,
        ):
            # ---- constants ----
            id16 = cpool.tile([128, 128], dt.float16, tag="id16")
            make_identity(nc, id16[:])
            ones1 = cpool.tile([1, 128], dt.float16, tag="ones1")
            nc.vector.memset(ones1[:], 1.0)
            onesbf = cpool.tile([128, 128], dt.bfloat16, tag="onesbf")
            nc.vector.memset(onesbf[:], 1.0)
            bq_sb = cpool.tile([1, 3 * JW], dt.float16, tag="bq")
            nc.sync.dma_start(bq_sb[:], bqkv[:])

            # ---- resident tensors ----
            QT = [rpool.tile([128, S_TOT], dt.float16, tag=f"qt{h}", name=f"qt{h}") for h in range(HL)]
            KT = [rpool.tile([128, S_TOT], dt.float16, tag=f"kt{h}", name=f"kt{h}") for h in range(HL)]
            OT = [rpool.tile([128, S_TOT], dt.float16, tag=f"ot{h}", name=f"ot{h}") for h in range(HL)]
            v_sb = rpool.tile([128, NB * 3 * HD], dt.bfloat16, tag="vsb")
            v_v = v_sb[:].rearrange("p (b j) -> p b j", b=NB)  # [128, 20, 384]

            streams = [
                (NB_ENC, 0, eT_r, waqkv_r, True),
                (NB_IMG, NB_ENC, hT_r, wqkv_r, False),
            ]

            # ================= Phase A (own psum pool: 6 banks) ==========
            psA_ctx = tc.tile_pool(name="psA", bufs=2, space="PSUM")
            pspool = psA_ctx.__enter__()
            # HAM warmup: keep PE busy during the initial weight/act DMAs so
            # the clock gate reaches 2.4 GHz before the first real matmul.
            warm = pspool.tile([128, CH], dt.float32, tag="psC")
            for _ in range(40):
                nc.tensor.matmul(warm[:, 0:128], id16[:], id16[:],
                                 start=True, stop=True)
            # ---- Phase A-qk ----
            for nblk, blk0, x_r, w_r, has_bias in streams:
                w_t = wpool.tile([128, NKK * 2 * JW], dt.float16, tag="w")
                w_v = w_t[:].rearrange("p (kk j) -> p kk j", kk=NKK)
                nc.sync.dma_start(w_v[:, 0:6, :], w_r[:, 0:6, 0 : 2 * JW])
                for tb in range(nblk):
                    gtb = blk0 + tb
                    xt = xpool.tile([128, NKK * 128], dt.float16, tag="xt")
                    xt_v = xt[:].rearrange("p (kk t) -> p kk t", kk=NKK)
                    nc.sync.dma_start(xt_v, x_r[:, :, tb * 128 : tb * 128 + 128])
                    rsl = slice(gtb * 128, gtb * 128 + 128)
                    cq = tpool.tile([128, HD], dt.float16, tag="cq")
                    nc.sync.dma_start(cq[:], cosq[rsl, :])
                    sq_ = tpool.tile([128, HD], dt.float16, tag="sq")
                    nc.sync.dma_start(sq_[:], sinq[rsl, :])
                    ck = tpool.tile([128, HD], dt.float16, tag="ck")
                    nc.sync.dma_start(ck[:], cosk[rsl, :])
                    sk = tpool.tile([128, HD], dt.float16, tag="sk")
                    nc.sync.dma_start(sk[:], sink[rsl, :])
                    if tb == 0:
                        for g in range(1, 4):
                            gs = slice(g * 6, (g + 1) * 6)
                            nc.sync.dma_start(w_v[:, gs, :], w_r[:, gs, 0 : 2 * JW])

                    pq = pspool.tile([128, CH], dt.float32, tag="psA")
                    pk = pspool.tile([128, CH], dt.float32, tag="psB")
                    for kk in range(NKK):
                        st_f = kk == 0
                        sp_f = (kk == NKK - 1) and not has_bias
                        nc.tensor.matmul(pq[:, 0:JW], xt_v[:, kk], w_v[:, kk, 0:JW],
                                         start=st_f, stop=sp_f)
                        nc.tensor.matmul(pk[:, 0:JW], xt_v[:, kk], w_v[:, kk, JW : 2 * JW],
                                         start=st_f, stop=sp_f)
                    if has_bias:
                        nc.tensor.matmul(pq[:, 0:JW], ones1[:], bq_sb[:, 0:JW],
                                         start=False, stop=True)
                        nc.tensor.matmul(pk[:, 0:JW], ones1[:], bq_sb[:, JW : 2 * JW],
                                         start=False, stop=True)

                    qsb = apool.tile([128, JW], dt.float32, tag="qsb")
                    nc.scalar.copy(qsb[:], pq[:, 0:JW])
                    ksb = apool.tile([128, JW], dt.float32, tag="ksb")
                    nc.scalar.copy(ksb[:], pk[:, 0:JW])

                    junk = apool.tile([128, HD], dt.float32, tag="junk")
                    ssq = apool.tile([128, 2 * HL], dt.float32, tag="ssq")
                    for h in range(HL):
                        nc.scalar.activation(junk[:], pq[:, ts(h, HD)], AF.Square,
                                             accum_out=ssq[:, h : h + 1])
                    for h in range(HL):
                        nc.scalar.activation(junk[:], pk[:, ts(h, HD)], AF.Square,
                                             accum_out=ssq[:, HL + h : HL + h + 1])
                    # inv = rsqrt(ssq/HD + eps): bit-trick seed + 2 Newton iters
                    ms = apool.tile([128, 2 * HL], dt.float32, tag="ms")
                    nc.vector.tensor_scalar(ms[:], ssq[:], 1.0 / HD, EPS,
                                            op0=mult, op1=AT.add)
                    sh_t = apool.tile([128, 2 * HL], dt.int32, tag="sh_t")
                    nc.vector.tensor_scalar(sh_t[:], ms[:].bitcast(dt.int32), 1, -1,
                                            op0=AT.logical_shift_right,
                                            op1=AT.bitwise_xor)
                    ya = apool.tile([128, 2 * HL], dt.float32, tag="ya")
                    nc.vector.tensor_scalar(ya[:].bitcast(dt.int32), sh_t[:],
                                            MAGIC1, None, op0=AT.add)
                    nta = apool.tile([128, 2 * HL], dt.float32, tag="nta")
                    ntc = apool.tile([128, 2 * HL], dt.float32, tag="ntc")
                    inv = apool.tile([128, 2 * HL], dt.float32, tag="inv")
                    for ysrc, ydst in ((ya, inv), (inv, ya)):
                        nc.vector.tensor_mul(nta[:], ysrc[:], ysrc[:])
                        nc.vector.tensor_mul(nta[:], nta[:], ms[:])
                        nc.vector.tensor_scalar(ntc[:], nta[:], -0.5, 1.5,
                                                op0=mult, op1=AT.add)
                        nc.vector.tensor_mul(ydst[:], ysrc[:], ntc[:])
                    inv = ya

                    for src, dstT, ct, st_t, iv0 in (
                        (qsb, QT, cq, sq_, 0),
                        (ksb, KT, ck, sk, HL),
                    ):
                        t1 = apool.tile([128, JW], dt.float32, tag="t1")
                        t2 = apool.tile([128, JW], dt.float32, tag="t2")
                        st_p = st_t[:].rearrange("p (a b) -> p a b", b=2)
                        for h in range(HL):
                            iv = inv[:, iv0 + h : iv0 + h + 1]
                            hs = slice(h * HD, (h + 1) * HD)
                            nc.vector.scalar_tensor_tensor(
                                t1[:, hs], src[:, hs], iv, ct[:], op0=mult, op1=mult)
                            srev = src[:, hs].rearrange(
                                "p (a b) -> p a b", b=2)[:, :, ::-1]
                            t2p = t2[:, hs].rearrange("p (a b) -> p a b", b=2)
                            nc.vector.scalar_tensor_tensor(
                                t2p, srev, iv, st_p, op0=mult, op1=mult)
                        qf = apool.tile([128, JW], dt.float16, tag="qf")
                        nc.vector.tensor_add(qf[:], t1[:], t2[:])
                        for h in range(HL):
                            ptr = pspool.tile([128, CH], dt.float16, tag="psC")
                            nc.tensor.transpose(ptr[:, 0:128], qf[:, ts(h, HD)], id16[:])
                            nc.any.tensor_copy(dstT[h][:, gtb * 128 : gtb * 128 + 128],
                                               ptr[:, 0:128])

            # ---- Phase A-v ----
            for nblk, blk0, x_r, w_r, has_bias in streams:
                wv_t = wpool.tile([128, NKK * JW], dt.float16, tag="w")
                wv_v = wv_t[:].rearrange("p (kk j) -> p kk j", kk=NKK)
                nc.sync.dma_start(wv_v, w_r[:, :, 2 * JW : 3 * JW])
                for tb in range(nblk):
                    gtb = blk0 + tb
                    xt = xpool.tile([128, NKK * 128], dt.float16, tag="xt")
                    xt_v = xt[:].rearrange("p (kk t) -> p kk t", kk=NKK)
                    nc.sync.dma_start(xt_v, x_r[:, :, tb * 128 : tb * 128 + 128])
                    pv = pspool.tile([128, CH], dt.float32, tag="psA")  # shares pq slots
                    for kk in range(NKK):
                        nc.tensor.matmul(pv[:, 0:JW], xt_v[:, kk], wv_v[:, kk],
                                         start=(kk == 0),
                                         stop=(kk == NKK - 1) and not has_bias)
                    if has_bias:
                        nc.tensor.matmul(pv[:, 0:JW], ones1[:], bq_sb[:, 2 * JW : 3 * JW],
                                         start=False, stop=True)
                    nc.vector.tensor_copy(v_v[:, gtb, :], pv[:, 0:JW])

            psA_ctx.__exit__(None, None, None)

            # ============ Phases B + C (own psum pool: 8 banks) ==========
            psB_ctx = tc.tile_pool(name="psB", bufs=1, space="PSUM")
            pspool = psB_ctx.__enter__()
            wo_t = wpool.tile([128, HL * D], dt.float16, tag="w")
            wo_v = wo_t[:].rearrange("p (j m) -> p j m", j=HL)
            nc.sync.dma_start(wo_v, wo_r[:])
            wadd_t = wpool.tile([128, HL * D], dt.float16, tag="w")
            wadd_v = wadd_t[:].rearrange("p (j m) -> p j m", j=HL)
            nc.sync.dma_start(wadd_v, wadd_r[:])

            for chunk in range(NCHUNK):
                qsl = slice(chunk * CH, (chunk + 1) * CH)
                for h in range(HL):
                    u = pspool.tile([128, CH], dt.float32, tag="u", bufs=2)
                    sm = pspool.tile([128, CH], dt.float32, tag="sm", bufs=2)
                    for b in range(NB):
                        st_ = pspool.tile([128, CH], dt.float32, tag="st", bufs=2)
                        pt = bpool.tile([128, CH], dt.bfloat16, tag="pt")
                        nc.tensor.matmul(st_[:], KT[h][:, ts(b, 128)],
                                         QT[h][:, qsl], start=True, stop=True)
                        nc.scalar.activation(pt[:], st_[:], AF.Exp, scale=SCALE)
                        nc.tensor.matmul(u[:], v_v[:, b, ts(h, HD)], pt[:],
                                         start=(b == 0), stop=(b == NB - 1))
                        nc.tensor.matmul(sm[:], onesbf[:], pt[:],
                                         start=(b == 0), stop=(b == NB - 1))
                    invb = bpool.tile([128, CH], dt.float32, tag="invb")
                    nc.vector.reciprocal_approx_fast(invb[:], sm[:])
                    nc.vector.tensor_mul(OT[h][:, qsl], u[:], invb[:])

                # output projection for this chunk's 4 token blocks
                w_sel = wadd_v if chunk == 0 else wo_v
                for tb in range(4):
                    gtb = chunk * 4 + tb
                    csl = slice(gtb * 128, gtb * 128 + 128)
                    for mc in range(MCH):
                        po = pspool.tile([128, CH], dt.float32, tag="po", bufs=2)
                        for j in range(HL):
                            nc.tensor.matmul(po[:], OT[j][:, csl],
                                             w_sel[:, j, ts(mc, CH)],
                                             start=(j == 0), stop=(j == HL - 1))
                        osb = opool.tile([128, CH], dt.float16, tag="osb")
                        nc.any.tensor_copy(osb[:], po[:])
                        if chunk == 0:
                            nc.sync.dma_start(enc_p[csl, ts(mc, CH)], osb[:])
                        else:
                            isl = slice(gtb * 128 - S_ENC, gtb * 128 - S_ENC + 128)
                            nc.sync.dma_start(img_p[isl, ts(mc, CH)], osb[:])

            psB_ctx.__exit__(None, None, None)

    nc.finalize()
    return nc


def _get_program():
    if "nc" not in _CACHE:
        _CACHE["nc"] = _build()
    return _CACHE["nc"]


def _prep_tables(cos, sin, g_enc, g_img):
    """Fold per-head gains and rotation signs into cos/sin tables.

    out = (x*inv*g)*cos + rot(x*inv*g)*sin with rot(x)[2i] = -x[2i+1],
    rot(x)[2i+1] = x[2i].  We apply (x*inv) on device, so:
      cos_t[t, d]   = cos[t, d] * g[d]
      sin_t[t, 2i]  = -sin[t, 2i]   * g[2i+1]   (multiplies x[2i+1])
      sin_t[t, 2i+1] = sin[t, 2i+1] * g[2i]     (multiplies x[2i])
    g = g_enc for t < S_ENC else g_img.
    """
    g = np.empty((S_TOT, HD), np.float32)
    g[:S_ENC] = np.asarray(g_enc, np.float32)[None, :]
    g[S_ENC:] = np.asarray(g_img, np.float32)[None, :]
    cos_t = (np.asarray(cos, np.float32) * g).astype(np.float32)
    sin_t = np.empty((S_TOT, HD), np.float32)
    sin_t[:, 0::2] = -np.asarray(sin, np.float32)[:, 0::2] * g[:, 1::2]
    sin_t[:, 1::2] = np.asarray(sin, np.float32)[:, 1::2] * g[:, 0::2]
    return (np.ascontiguousarray(cos_t).astype(np.float16),
            np.ascontiguousarray(sin_t).astype(np.float16))


def kernel(hidden_states, encoder_hidden_states, cos, sin,
           Wq, Wk, Wv, gq, gk,
           Waq, baq, Wak, bak, Wav, bav, gaq, gak,
           Wo, bo, Wadd, badd):
    from concourse.bass_utils import run_bass_kernel_spmd

    f32 = np.float32
    h = np.asarray(hidden_states, f32).reshape(S_IMG, D)
    e = np.asarray(encoder_hidden_states, f32).reshape(S_ENC, D)
    hT16 = np.ascontiguousarray(h.T).astype(np.float16)
    eT16 = np.ascontiguousarray(e.T).astype(np.float16)

    cos_q, sin_q = _prep_tables(cos, sin, gaq, gq)
    cos_k, sin_k = _prep_tables(cos, sin, gak, gk)

    Wq, Wk, Wv = (np.asarray(x, f32) for x in (Wq, Wk, Wv))
    Waq, Wak, Wav = (np.asarray(x, f32) for x in (Waq, Wak, Wav))
    Wo_, Wadd_ = np.asarray(Wo, f32), np.asarray(Wadd, f32)
    baq, bak, bav = (np.asarray(x, f32) for x in (baq, bak, bav))

    in_maps = []
    for c in range(NCORES):
        J = slice(c * JW, (c + 1) * JW)
        wqkv_c = np.concatenate([Wq[J].T, Wk[J].T, Wv[J].T], axis=1).astype(np.float16)
        waqkv_c = np.concatenate([Waq[J].T, Wak[J].T, Wav[J].T], axis=1).astype(np.float16)
        bqkv_c = np.concatenate([baq[J], bak[J], bav[J]])[None, :].astype(np.float16)
        wo_c = np.ascontiguousarray(Wo_[:, J].T).astype(np.float16)
        wadd_c = np.ascontiguousarray(Wadd_[:, J].T).astype(np.float16)
        in_maps.append({
            "hT": hT16, "eT": eT16,
            "wqkv": np.ascontiguousarray(wqkv_c),
            "waqkv": np.ascontiguousarray(waqkv_c),
            "bqkv": bqkv_c,
            "cosq": cos_q, "sinq": sin_q, "cosk": cos_k, "sink": sin_k,
            "wo": wo_c, "wadd": wadd_c,
        })

    nc = _get_program()
    trace = bool(int(os.environ.get("KERNEL_TRACE", "0")))
    res = run_bass_kernel_spmd(nc, in_maps, core_ids=list(range(NCORES)),
                               trace=trace)
    _CACHE["last_results"] = res

    img = np.zeros((S_IMG, D), f32)
    enc = np.zeros((S_ENC, D), f32)
    for c in range(NCORES):
        img += res.results[c]["img_p"]
        enc += res.results[c]["enc_p"]
    img += np.asarray(bo, f32)[None, :]
    enc += np.asarray(badd, f32)[None, :]
    return img.reshape(B, S_IMG, D), enc.reshape(B, S_ENC, D)


# revision 21
# speedup vs baseline: 1.0635x; 1.0258x over previous
"""Trainium2 Bass kernel for nn_Attention_63075889709156 (joint img/text attention).

Strategy: tensor-parallel over heads across 8 NeuronCores (3 heads each).
Per core:
  A) QKV projections (fp16 matmuls, fp32 PSUM accum) for both streams,
     fused per-head RMSNorm (ACT Square+accum, ln/exp rsqrt) and RoPE
     (gains + rotation signs folded into host-prepared cos/sin tables,
     applied with fused scalar_tensor_tensor ops), PE transposes to get
     Q^T/K^T [hd, seq] fp16; V kept natural [seq, hd] bf16.
  B) Attention per (head, 512-query chunk): scores computed transposed
     [tk, tq] so ACT exp output feeds the P@V matmul directly; row-sums
     via an all-ones stationary matmul (gives broadcast for free);
     normalize with reciprocal_approx_fast; out^T [hd, tq] fp16.
  C) Output projections vs column-slices of Wo/Wadd -> per-core partial
     [seq, 3072] fp32, summed across cores on the host (all-reduce
     equivalent), biases added on host.

kernel(**inputs) takes the FULL unsharded inputs and returns the full
(img_out, enc_out) tuple, matching reference.reference().
"""
import os
import sys

sys.path.insert(0, "/opt/trn_rl_repo")

import numpy as np

B, S_IMG, S_ENC, D, H_TOT, HD = 1, 2048, 512, 3072, 24, 128
S_TOT = S_IMG + S_ENC  # 2560
NCORES = 8
HL = H_TOT // NCORES   # 3 heads per core
JW = HL * HD           # 384 output cols per matrix per core
NKK = D // 128         # 24 contraction tiles
NB_ENC = S_ENC // 128  # 4
NB_IMG = S_IMG // 128  # 16
NB = NB_ENC + NB_IMG   # 20
CH = 512               # query-chunk / free-dim tile
NCHUNK = S_TOT // CH   # 5 (chunk 0 == the encoder tokens exactly)
MCH = D // CH          # 6 output column chunks
EPS = 1e-5
SCALE = 1.0 / float(np.sqrt(HD))

_CACHE = {}


def _build():
    import concourse.bass as bass
    import concourse.mybir as mybir
    import concourse.tile as tile
    from concourse import bacc
    from concourse.masks import make_identity

    dt = mybir.dt
    ts = bass.ts
    mult = mybir.AluOpType.mult
    AT = mybir.AluOpType
    MAGIC1 = 0x5F3759DF + 1
    AF = mybir.ActivationFunctionType

    nc = bacc.Bacc(trn_type="TRN2", target_bir_lowering=False)

    hT = nc.dram_tensor("hT", [D, S_IMG], dt.float16, kind="ExternalInput")
    eT = nc.dram_tensor("eT", [D, S_ENC], dt.float16, kind="ExternalInput")
    wqkv = nc.dram_tensor("wqkv", [D, 3 * JW], dt.float16, kind="ExternalInput")
    waqkv = nc.dram_tensor("waqkv", [D, 3 * JW], dt.float16, kind="ExternalInput")
    bqkv = nc.dram_tensor("bqkv", [1, 3 * JW], dt.float16, kind="ExternalInput")
    cosq = nc.dram_tensor("cosq", [S_TOT, HD], dt.float16, kind="ExternalInput")
    sinq = nc.dram_tensor("sinq", [S_TOT, HD], dt.float16, kind="ExternalInput")
    cosk = nc.dram_tensor("cosk", [S_TOT, HD], dt.float16, kind="ExternalInput")
    sink = nc.dram_tensor("sink", [S_TOT, HD], dt.float16, kind="ExternalInput")
    wo = nc.dram_tensor("wo", [JW, D], dt.float16, kind="ExternalInput")
    wadd = nc.dram_tensor("wadd", [JW, D], dt.float16, kind="ExternalInput")

    img_p = nc.dram_tensor("img_p", [S_IMG, D], dt.float16, kind="ExternalOutput")
    enc_p = nc.dram_tensor("enc_p", [S_ENC, D], dt.float16, kind="ExternalOutput")

    # partition-tiled DRAM views: [(kk p) n] -> [p kk n]
    hT_r = hT.rearrange("(kk p) t -> p kk t", p=128)
    eT_r = eT.rearrange("(kk p) t -> p kk t", p=128)
    wqkv_r = wqkv.rearrange("(kk p) j -> p kk j", p=128)
    waqkv_r = waqkv.rearrange("(kk p) j -> p kk j", p=128)
    wo_r = wo.rearrange("(j p) m -> p j m", p=128)      # [128, 3, 3072]
    wadd_r = wadd.rearrange("(j p) m -> p j m", p=128)

    with tile.TileContext(nc) as tc:
        with (
            tc.tile_pool(name="cpool", bufs=1) as cpool,
            tc.tile_pool(name="rpool", bufs=1) as rpool,
            tc.tile_pool(name="wpool", bufs=2) as wpool,
            tc.tile_pool(name="xpool", bufs=3) as xpool,
            tc.tile_pool(name="tpool", bufs=2) as tpool,
            tc.tile_pool(name="apool", bufs=3) as apool,
            tc.tile_pool(name="bpool", bufs=4) as bpool,
            tc.tile_pool(name="opool", bufs=3) as opool,
        ):
            # ---- constants ----
            id16 = cpool.tile([128, 128], dt.float16, tag="id16")
            make_identity(nc, id16[:])
            ones1 = cpool.tile([1, 128], dt.float16, tag="ones1")
            nc.vector.memset(ones1[:], 1.0)
            onesbf = cpool.tile([128, 128], dt.bfloat16, tag="onesbf")
            nc.vector.memset(onesbf[:], 1.0)
            bq_sb = cpool.tile([1, 3 * JW], dt.float16, tag="bq")
            nc.sync.dma_start(bq_sb[:], bqkv[:])

            # ---- resident tensors ----
            QT = [rpool.tile([128, S_TOT], dt.float16, tag=f"qt{h}", name=f"qt{h}") for h in range(HL)]
            KT = [rpool.tile([128, S_TOT], dt.float16, tag=f"kt{h}", name=f"kt{h}") for h in range(HL)]
            OT = [rpool.tile([128, S_TOT], dt.float16, tag=f"ot{h}", name=f"ot{h}") for h in range(HL)]
            v_sb = rpool.tile([128, NB * 3 * HD], dt.bfloat16, tag="vsb")
            v_v = v_sb[:].rearrange("p (b j) -> p b j", b=NB)  # [128, 20, 384]

            streams = [
                (NB_ENC, 0, eT_r, waqkv_r, True),
                (NB_IMG, NB_ENC, hT_r, wqkv_r, False),
            ]

            # ================= Phase A (own psum pool: 6 banks) ==========
            psA_ctx = tc.tile_pool(name="psA", bufs=2, space="PSUM")
            pspool = psA_ctx.__enter__()
            # HAM warmup: keep PE busy during the initial weight/act DMAs so
            # the clock gate reaches 2.4 GHz before the first real matmul.
            warm = pspool.tile([128, CH], dt.float32, tag="psC")
            for _ in range(40):
                nc.tensor.matmul(warm[:, 0:128], id16[:], id16[:],
                                 start=True, stop=True)
            # ---- Phase A-qk ----
            for nblk, blk0, x_r, w_r, has_bias in streams:
                w_t = wpool.tile([128, NKK * 2 * JW], dt.float16, tag="w")
                w_v = w_t[:].rearrange("p (kk j) -> p kk j", kk=NKK)
                nc.sync.dma_start(w_v[:, 0:6, :], w_r[:, 0:6, 0 : 2 * JW])
                for tb in range(nblk):
                    gtb = blk0 + tb
                    xt = xpool.tile([128, NKK * 128], dt.float16, tag="xt")
                    xt_v = xt[:].rearrange("p (kk t) -> p kk t", kk=NKK)
                    nc.sync.dma_start(xt_v, x_r[:, :, tb * 128 : tb * 128 + 128])
                    rsl = slice(gtb * 128, gtb * 128 + 128)
                    cq = tpool.tile([128, HD], dt.float16, tag="cq")
                    nc.gpsimd.dma_start(cq[:], cosq[rsl, :])
                    sq_ = tpool.tile([128, HD], dt.float16, tag="sq")
                    nc.gpsimd.dma_start(sq_[:], sinq[rsl, :])
                    ck = tpool.tile([128, HD], dt.float16, tag="ck")
                    nc.gpsimd.dma_start(ck[:], cosk[rsl, :])
                    sk = tpool.tile([128, HD], dt.float16, tag="sk")
                    nc.gpsimd.dma_start(sk[:], sink[rsl, :])
                    if tb == 0:
                        for g in range(1, 4):
                            gs = slice(g * 6, (g + 1) * 6)
                            nc.sync.dma_start(w_v[:, gs, :], w_r[:, gs, 0 : 2 * JW])

                    pq = pspool.tile([128, CH], dt.float32, tag="psA")
                    pk = pspool.tile([128, CH], dt.float32, tag="psB")
                    for kk in range(NKK):
                        st_f = kk == 0
                        sp_f = (kk == NKK - 1) and not has_bias
                        nc.tensor.matmul(pq[:, 0:JW], xt_v[:, kk], w_v[:, kk, 0:JW],
                                         start=st_f, stop=sp_f)
                        nc.tensor.matmul(pk[:, 0:JW], xt_v[:, kk], w_v[:, kk, JW : 2 * JW],
                                         start=st_f, stop=sp_f)
                    if has_bias:
                        nc.tensor.matmul(pq[:, 0:JW], ones1[:], bq_sb[:, 0:JW],
                                         start=False, stop=True)
                        nc.tensor.matmul(pk[:, 0:JW], ones1[:], bq_sb[:, JW : 2 * JW],
                                         start=False, stop=True)

                    qsb = apool.tile([128, JW], dt.float32, tag="qsb")
                    nc.scalar.copy(qsb[:], pq[:, 0:JW])
                    ksb = apool.tile([128, JW], dt.float32, tag="ksb")
                    nc.scalar.copy(ksb[:], pk[:, 0:JW])

                    junk = apool.tile([128, HD], dt.float32, tag="junk")
                    ssq = apool.tile([128, 2 * HL], dt.float32, tag="ssq")
                    for h in range(HL):
                        nc.scalar.activation(junk[:], pq[:, ts(h, HD)], AF.Square,
                                             accum_out=ssq[:, h : h + 1])
                    for h in range(HL):
                        nc.scalar.activation(junk[:], pk[:, ts(h, HD)], AF.Square,
                                             accum_out=ssq[:, HL + h : HL + h + 1])
                    # inv = rsqrt(ssq/HD + eps): bit-trick seed + 2 Newton iters
                    ms = apool.tile([128, 2 * HL], dt.float32, tag="ms")
                    nc.vector.tensor_scalar(ms[:], ssq[:], 1.0 / HD, EPS,
                                            op0=mult, op1=AT.add)
                    sh_t = apool.tile([128, 2 * HL], dt.int32, tag="sh_t")
                    nc.vector.tensor_scalar(sh_t[:], ms[:].bitcast(dt.int32), 1, -1,
                                            op0=AT.logical_shift_right,
                                            op1=AT.bitwise_xor)
                    ya = apool.tile([128, 2 * HL], dt.float32, tag="ya")
                    nc.vector.tensor_scalar(ya[:].bitcast(dt.int32), sh_t[:],
                                            MAGIC1, None, op0=AT.add)
                    nta = apool.tile([128, 2 * HL], dt.float32, tag="nta")
                    ntc = apool.tile([128, 2 * HL], dt.float32, tag="ntc")
                    inv = apool.tile([128, 2 * HL], dt.float32, tag="inv")
                    for ysrc, ydst in ((ya, inv), (inv, ya)):
                        nc.vector.tensor_mul(nta[:], ysrc[:], ysrc[:])
                        nc.vector.tensor_mul(nta[:], nta[:], ms[:])
                        nc.vector.tensor_scalar(ntc[:], nta[:], -0.5, 1.5,
                                                op0=mult, op1=AT.add)
                        nc.vector.tensor_mul(ydst[:], ysrc[:], ntc[:])
                    inv = ya

                    for src, dstT, ct, st_t, iv0 in (
                        (qsb, QT, cq, sq_, 0),
                        (ksb, KT, ck, sk, HL),
                    ):
                        t1 = apool.tile([128, JW], dt.float32, tag="t1")
                        t2 = apool.tile([128, JW], dt.float32, tag="t2")
                        st_p = st_t[:].rearrange("p (a b) -> p a b", b=2)
                        for h in range(HL):
                            iv = inv[:, iv0 + h : iv0 + h + 1]
                            hs = slice(h * HD, (h + 1) * HD)
                            nc.vector.scalar_tensor_tensor(
                                t1[:, hs], src[:, hs], iv, ct[:], op0=mult, op1=mult)
                            srev = src[:, hs].rearrange(
                                "p (a b) -> p a b", b=2)[:, :, ::-1]
                            t2p = t2[:, hs].rearrange("p (a b) -> p a b", b=2)
                            nc.vector.scalar_tensor_tensor(
                                t2p, srev, iv, st_p, op0=mult, op1=mult)
                        qf = apool.tile([128, JW], dt.float16, tag="qf")
                        nc.vector.tensor_add(qf[:], t1[:], t2[:])
                        for h in range(HL):
                            ptr = pspool.tile([128, CH], dt.float16, tag="psC")
                            nc.tensor.transpose(ptr[:, 0:128], qf[:, ts(h, HD)], id16[:])
                            nc.any.tensor_copy(dstT[h][:, gtb * 128 : gtb * 128 + 128],
                                               ptr[:, 0:128])

            # ---- Phase A-v ----
            for nblk, blk0, x_r, w_r, has_bias in streams:
                wv_t = wpool.tile([128, NKK * JW], dt.float16, tag="w")
                wv_v = wv_t[:].rearrange("p (kk j) -> p kk j", kk=NKK)
                nc.sync.dma_start(wv_v, w_r[:, :, 2 * JW : 3 * JW])
                for tb in range(nblk):
                    gtb = blk0 + tb
                    xt = xpool.tile([128, NKK * 128], dt.float16, tag="xt")
                    xt_v = xt[:].rearrange("p (kk t) -> p kk t", kk=NKK)
                    nc.sync.dma_start(xt_v, x_r[:, :, tb * 128 : tb * 128 + 128])
                    pv = pspool.tile([128, CH], dt.float32, tag="psA")  # shares pq slots
                    for kk in range(NKK):
                        nc.tensor.matmul(pv[:, 0:JW], xt_v[:, kk], wv_v[:, kk],
                                         start=(kk == 0),
                                         stop=(kk == NKK - 1) and not has_bias)
                    if has_bias:
                        nc.tensor.matmul(pv[:, 0:JW], ones1[:], bq_sb[:, 2 * JW : 3 * JW],
                                         start=False, stop=True)
                    nc.vector.tensor_copy(v_v[:, gtb, :], pv[:, 0:JW])

            psA_ctx.__exit__(None, None, None)

            # ============ Phases B + C (own psum pool: 8 banks) ==========
            psB_ctx = tc.tile_pool(name="psB", bufs=1, space="PSUM")
            pspool = psB_ctx.__enter__()
            wadd_t = wpool.tile([128, HL * D], dt.float16, tag="w")
            wadd_v = wadd_t[:].rearrange("p (j m) -> p j m", j=HL)
            nc.sync.dma_start(wadd_v, wadd_r[:])
            wo_t = wpool.tile([128, HL * D], dt.float16, tag="w")
            wo_v = wo_t[:].rearrange("p (j m) -> p j m", j=HL)
            nc.sync.dma_start(wo_v, wo_r[:])

            for chunk in range(NCHUNK):
                qsl = slice(chunk * CH, (chunk + 1) * CH)
                for h in range(HL):
                    u = pspool.tile([128, CH], dt.float32, tag="u", bufs=2)
                    sm = pspool.tile([128, CH], dt.float32, tag="sm", bufs=2)
                    for b in range(NB):
                        st_ = pspool.tile([128, CH], dt.float32, tag="st", bufs=2)
                        pt = bpool.tile([128, CH], dt.bfloat16, tag="pt")
                        nc.tensor.matmul(st_[:], KT[h][:, ts(b, 128)],
                                         QT[h][:, qsl], start=True, stop=True)
                        nc.scalar.activation(pt[:], st_[:], AF.Exp, scale=SCALE)
                        nc.tensor.matmul(u[:], v_v[:, b, ts(h, HD)], pt[:],
                                         start=(b == 0), stop=(b == NB - 1))
                        nc.tensor.matmul(sm[:], onesbf[:], pt[:],
                                         start=(b == 0), stop=(b == NB - 1))
                    invb = bpool.tile([128, CH], dt.float32, tag="invb")
                    nc.vector.reciprocal_approx_fast(invb[:], sm[:])
                    nc.vector.tensor_mul(OT[h][:, qsl], u[:], invb[:])

                # output projection for this chunk's 4 token blocks
                w_sel = wadd_v if chunk == 0 else wo_v
                for tb in range(4):
                    gtb = chunk * 4 + tb
                    csl = slice(gtb * 128, gtb * 128 + 128)
                    for mc in range(MCH):
                        po = pspool.tile([128, CH], dt.float32, tag="po", bufs=2)
                        for j in range(HL):
                            nc.tensor.matmul(po[:], OT[j][:, csl],
                                             w_sel[:, j, ts(mc, CH)],
                                             start=(j == 0), stop=(j == HL - 1))
                        osb = opool.tile([128, CH], dt.float16, tag="osb")
                        nc.any.tensor_copy(osb[:], po[:])
                        if chunk == 0:
                            nc.sync.dma_start(enc_p[csl, ts(mc, CH)], osb[:])
                        else:
                            isl = slice(gtb * 128 - S_ENC, gtb * 128 - S_ENC + 128)
                            nc.sync.dma_start(img_p[isl, ts(mc, CH)], osb[:])

            psB_ctx.__exit__(None, None, None)

    nc.finalize()
    return nc


def _get_program():
    if "nc" not in _CACHE:
        _CACHE["nc"] = _build()
    return _CACHE["nc"]


def _prep_tables(cos, sin, g_enc, g_img):
    """Fold per-head gains and rotation signs into cos/sin tables.

    out = (x*inv*g)*cos + rot(x*inv*g)*sin with rot(x)[2i] = -x[2i+1],
    rot(x)[2i+1] = x[2i].  We apply (x*inv) on device, so:
      cos_t[t, d]   = cos[t, d] * g[d]
      sin_t[t, 2i]  = -sin[t, 2i]   * g[2i+1]   (multiplies x[2i+1])
      sin_t[t, 2i+1] = sin[t, 2i+1] * g[2i]     (multiplies x[2i])
    g = g_enc for t < S_ENC else g_img.
    """
    g = np.empty((S_TOT, HD), np.float32)
    g[:S_ENC] = np.asarray(g_enc, np.float32)[None, :]
    g[S_ENC:] = np.asarray(g_img, np.float32)[None, :]
    cos_t = (np.asarray(cos, np.float32) * g).astype(np.float32)
    sin_t = np.empty((S_TOT, HD), np.float32)
    sin_t[:, 0::2] = -np.asarray(sin, np.float32)[:, 0::2] * g[:, 1::2]
    sin_t[:, 1::2] = np.asarray(sin, np.float32)[:, 1::2] * g[:, 0::2]
    return (np.ascontiguousarray(cos_t).astype(np.float16),
            np.ascontiguousarray(sin_t).astype(np.float16))


def kernel(hidden_states, encoder_hidden_states, cos, sin,
           Wq, Wk, Wv, gq, gk,
           Waq, baq, Wak, bak, Wav, bav, gaq, gak,
           Wo, bo, Wadd, badd):
    from concourse.bass_utils import run_bass_kernel_spmd

    f32 = np.float32
    h = np.asarray(hidden_states, f32).reshape(S_IMG, D)
    e = np.asarray(encoder_hidden_states, f32).reshape(S_ENC, D)
    hT16 = np.ascontiguousarray(h.T).astype(np.float16)
    eT16 = np.ascontiguousarray(e.T).astype(np.float16)

    cos_q, sin_q = _prep_tables(cos, sin, gaq, gq)
    cos_k, sin_k = _prep_tables(cos, sin, gak, gk)

    Wq, Wk, Wv = (np.asarray(x, f32) for x in (Wq, Wk, Wv))
    Waq, Wak, Wav = (np.asarray(x, f32) for x in (Waq, Wak, Wav))
    Wo_, Wadd_ = np.asarray(Wo, f32), np.asarray(Wadd, f32)
    baq, bak, bav = (np.asarray(x, f32) for x in (baq, bak, bav))

    in_maps = []
    for c in range(NCORES):
        J = slice(c * JW, (c + 1) * JW)
        wqkv_c = np.concatenate([Wq[J].T, Wk[J].T, Wv[J].T], axis=1).astype(np.float16)
        waqkv_c = np.concatenate([Waq[J].T, Wak[J].T, Wav[J].T], axis=1).astype(np.float16)
        bqkv_c = np.concatenate([baq[J], bak[J], bav[J]])[None, :].astype(np.float16)
        wo_c = np.ascontiguousarray(Wo_[:, J].T).astype(np.float16)
        wadd_c = np.ascontiguousarray(Wadd_[:, J].T).astype(np.float16)
        in_maps.append({
            "hT": hT16, "eT": eT16,
            "wqkv": np.ascontiguousarray(wqkv_c),
            "waqkv": np.ascontiguousarray(waqkv_c),
            "bqkv": bqkv_c,
            "cosq": cos_q, "sinq": sin_q, "cosk": cos_k, "sink": sin_k,
            "wo": wo_c, "wadd": wadd_c,
        })

    nc = _get_program()
    trace = bool(int(os.environ.get("KERNEL_TRACE", "0")))
    res = run_bass_kernel_spmd(nc, in_maps, core_ids=list(range(NCORES)),
                               trace=trace)
    _CACHE["last_results"] = res

    img = np.zeros((S_IMG, D), f32)
    enc = np.zeros((S_ENC, D), f32)
    for c in range(NCORES):
        img += res.results[c]["img_p"]
        enc += res.results[c]["enc_p"]
    img += np.asarray(bo, f32)[None, :]
    enc += np.asarray(badd, f32)[None, :]
    return img.reshape(B, S_IMG, D), enc.reshape(B, S_ENC, D)
